# revision 19
# baseline (speedup 1.0000x reference)
"""Multi-head distance (attention) layer on 8 TRN2 NeuronCores.

Sharding: data-parallel over batch. B=8 -> one batch element per core.
Each core computes a full multi-head self-attention for its [L=1024, D=256]
slice with H=8 heads of dim 64. No collectives needed.

Per-core algorithm (all layouts chosen so softmax needs no transposes and
all matmul operands are bf16 so the PE streams at 1 row/cycle):
  xT   = transpose(x)            (PE matmul against identity, ACT drains)
  qkT  = xT + peT                (pos-enc, host-precomputed constant, DVE)
  qT   = Wq.T @ x_pe             via matmul(lhsT=Wq, rhs=qkT)
  kTz  = Wk.T @ x_pe             per-head tiles, other head's rows zeroed
                                 (so S contracts K=128: K=64 runs half-rate)
  v    = x @ Wv                  via matmul(lhsT=xT, rhs=Wv)
  per head h:
    sT[m,l] = sum_d kTz[d,m] qT[d,l]     matmul, K=128 (zero-padded)
    eT      = exp(0.125 * sT)            ScalarE, PSUM->SBUF, bf16; S-chunks
                                         packed 3-per-PSUM-tile so each exp
                                         call is [128,1536]
    O[l,d]+Z = eT.T @ [v_h | 1]          matmul(lhsT=eT, rhs=v_aug), 4 output
                                         column-groups share one PSUM bank
    out_h   = O * (1/Z)                  DVE reciprocal + broadcast multiply
Bias handling: bq added to qT during PSUM drain (per-partition scalar, fp32
before bf16 rounding); bk only shifts each score row by a constant
(softmax-invariant) so it is dropped; bv shifts the output by exactly
repeat(bv, 64) because softmax rows sum to 1, added on the host.
"""

import numpy as np
import ml_dtypes

import concourse.bass as bass
import concourse.mybir as mybir
import concourse.tile as tile
from concourse import bacc
from concourse.bass_utils import run_bass_kernel_spmd
from concourse.masks import make_identity

B, L, D = 8, 1024, 256
H, HD = 8, 64
J = H * HD  # 512
TEMPERATURE = 10000.0

f32 = mybir.dt.float32
bf16 = mybir.dt.float16  # fp16: same PE rate as bf16, 8x the mantissa

_CACHE = {}
LAST_RESULT = None  # BassKernelResults of the most recent run (for profiling)
TRACE = False

STILE = 1024  # S-chunk PSUM/exp tile width (2 chunks of 512)


def _emit(tc, aps):
    nc = tc.nc
    Exp = mybir.ActivationFunctionType.Exp
    Copy = mybir.ActivationFunctionType.Copy
    x, wq, wk, wv, bqc, pet, out = (
        aps["x"], aps["wq"], aps["wk"], aps["wv"], aps["bqc"], aps["pet"], aps["out"],
    )

    xr = x.rearrange("(n p) c -> p n c", p=128)          # [128, 8, 256]
    petr = pet.rearrange("(t p) l -> t p l", p=128)      # [2, 128, 1024]
    wqr = wq.rearrange("(t p) j -> t p j", p=128)        # [2, 128, 512]
    wkr = wk.rearrange("(t p) j -> t p j", p=128)
    wvr = wv.rearrange("(t p) j -> t p j", p=128)
    outr = out.rearrange("(n p) j -> p n j", p=128)      # [128, 8, 512]

    import contextlib
    ctx = contextlib.ExitStack()
    persist = ctx.enter_context(tc.tile_pool(name="persist", bufs=1))
    epool = ctx.enter_context(tc.tile_pool(name="epool", bufs=18))
    rpool = ctx.enter_context(tc.tile_pool(name="rpool", bufs=4))
    s_ps = ctx.enter_context(tc.tile_pool(name="sps", bufs=3, space="PSUM"))
    o_ps = ctx.enter_context(tc.tile_pool(name="ops", bufs=2, space="PSUM"))

    # --- ACT exp-table preload (off the attention critical path) ---
    sc_in = persist.tile([128, 8], f32, name="sc_in")
    sc_out = persist.tile([128, 8], f32, name="sc_out")
    nc.vector.memset(sc_in[:], 0.0)
    nc.scalar.activation(sc_out[:], sc_in[:], Exp)

    kTz = [persist.tile([128, 1024], bf16, name=f"kTz{h}") for h in range(8)]
    for h in range(2):
        nc.vector.memset(kTz[h][:], 0.0)

    # --- input DMAs: x + wq on the SP HWDGE queue, rest on gpsimd SWDGE ---
    x_sb = persist.tile([128, 8, 256], bf16, name="x_sb")
    for qtr in range(4):
        nc.sync.dma_start(out=x_sb[:, qtr * 2:(qtr + 1) * 2, :],
                          in_=xr[:, qtr * 2:(qtr + 1) * 2, :])
    w_sb = {}
    for wname in ("wq", "wk", "wv"):
        w_sb[wname] = [
            persist.tile([128, 512], bf16, name=f"{wname}_sb{t}") for t in range(2)
        ]
    for t in range(2):
        nc.sync.dma_start(out=w_sb["wq"][t][:], in_=wqr[t])

    bq_sb = persist.tile([128, 4], f32, name="bq_sb")
    nc.gpsimd.dma_start(out=bq_sb[:], in_=bqc[:, :])
    pe_sb = [persist.tile([128, 1024], bf16, name=f"pe_sb{t}") for t in range(2)]
    for t in range(2):
        nc.gpsimd.dma_start(out=pe_sb[t][:], in_=petr[t])
    for wname, wr in (("wk", wkr), ("wv", wvr)):
        for t in range(2):
            nc.gpsimd.dma_start(out=w_sb[wname][t][:], in_=wr[t])

    ident = persist.tile([128, 128], bf16, name="ident")
    make_identity(nc, ident)

    # --- transpose x via PE (out = x_chunk.T @ I); 4 transposes packed per
    # PSUM tile, drains split between ScalarE and DVE ---
    xT = [persist.tile([128, 1024], bf16, name=f"xT{t}") for t in range(2)]
    for c2 in range(2):
        for g in range(2):  # n-groups of 4
            tp = s_ps.tile([128, STILE], f32, tag="s", name="tp")
            for i in range(4):
                n = 4 * g + i
                nc.tensor.matmul(
                    tp[:, i * 128:(i + 1) * 128],
                    lhsT=x_sb[:, n, c2 * 128:(c2 + 1) * 128],
                    rhs=ident[:],
                    start=True,
                    stop=True,
                )
            dst = xT[c2][:, g * 512:(g + 1) * 512]
            if g == 0:
                nc.scalar.activation(dst, tp[:, 0:512], Copy)
            else:
                nc.vector.tensor_copy(dst, tp[:, 0:512])

    qkT = [persist.tile([128, 1024], bf16, name=f"qkT{t}") for t in range(2)]
    for t in range(2):
        nc.vector.tensor_add(qkT[t][:], xT[t][:], pe_sb[t][:])

    # --- QKV projections (PSUM fills share the "s" tag slots) ---
    qT = [persist.tile([128, 1024], bf16, name=f"qT{j}") for j in range(4)]
    v_sb = [persist.tile([128, 8, 65], bf16, name=f"v_sb{m}") for m in range(8)]

    def qk_piece(j, which, l2):
        wname = "wq" if which == "q" else "wk"
        pq = s_ps.tile([128, STILE], f32, tag="s", name="pq")
        for c2 in range(2):
            nc.tensor.matmul(
                pq[:, 0:512],
                lhsT=w_sb[wname][c2][:, j * 128:(j + 1) * 128],
                rhs=qkT[c2][:, l2 * 512:(l2 + 1) * 512],
                start=(c2 == 0),
                stop=(c2 == 1),
            )
        dsl = slice(l2 * 512, (l2 + 1) * 512)
        if which == "q":
            nc.vector.tensor_scalar_add(
                qT[j][:, dsl], pq[:, 0:512], bq_sb[:, j:j + 1]
            )
        else:
            nc.vector.tensor_copy(kTz[2 * j][0:64, dsl], pq[0:64, 0:512])
            nc.vector.tensor_copy(kTz[2 * j + 1][64:128, dsl], pq[64:128, 0:512])

    def qk_proj(j, which):
        for l2 in range(2):
            qk_piece(j, which, l2)

    def v_proj(m):
        pv = s_ps.tile([128, STILE], f32, tag="s", name="pv")
        for c2 in range(2):
            nc.tensor.matmul(
                pv[:, 0:512],
                lhsT=xT[c2][:, m * 128:(m + 1) * 128],
                rhs=w_sb["wv"][c2][:],
                start=(c2 == 0),
                stop=(c2 == 1),
            )
        nc.vector.tensor_copy(
            v_sb[m][:, :, 0:64], pv[:, 0:512].rearrange("p (h d) -> p h d", h=8)
        )
        nc.vector.memset(v_sb[m][:, :, 64:65], 1.0)

    # --- attention: S-chunks packed into [128, STILE] PSUM tiles; one exp
    # per tile. Software-pipelined: S(h+1) emitted before O(h). ---
    out_sb = persist.tile([128, 8, 512], f32, name="out_sb")
    epos = {}  # (h, mc, l2) -> (e_tile, col_offset)
    state = {"tile": None, "off": 0, "chunks": []}

    def flush_exp():
        if state["tile"] is None or not state["chunks"]:
            return
        e = epool.tile([128, state["off"]], bf16, tag="e", name="e")
        nc.scalar.activation(
            e[:], state["tile"][:, 0:state["off"]], Exp, scale=float(HD) ** -0.5
        )
        for key, off in state["chunks"]:
            epos[key] = (e, off)
        state["tile"] = None
        state["off"] = 0
        state["chunks"] = []

    def s_chunk(h, mc, l2):
        if state["tile"] is None:
            state["tile"] = s_ps.tile([128, STILE], f32, tag="s", name="ps")
        off = state["off"]
        nc.tensor.matmul(
            state["tile"][:, off:off + 512],
            lhsT=kTz[h][:, mc * 128:(mc + 1) * 128],
            rhs=qT[h // 2][:, l2 * 512:(l2 + 1) * 512],
            start=True,
            stop=True,
        )
        state["chunks"].append(((h, mc, l2), off))
        state["off"] = off + 512
        if state["off"] == STILE:
            flush_exp()

    def emit_S_half(h, l2):
        for mc in range(8):
            s_chunk(h, mc, l2)

    def emit_O_quad(h, q):
        hsl = slice(h * 64, (h + 1) * 64)
        pO = o_ps.tile([128, 260], f32, tag="o", name="pO")
        for g in range(4):
            lc = 4 * q + g
            l2, sub = lc // 4, lc % 4
            for mc in range(8):
                e, off = epos[(h, mc, l2)]
                nc.tensor.matmul(
                    pO[:, 65 * g:65 * g + 65],
                    lhsT=e[:, off + sub * 128:off + (sub + 1) * 128],
                    rhs=v_sb[mc][:, h, :],
                    start=(mc == 0),
                    stop=(mc == 7),
                )
        pOr = pO.rearrange("p (g c) -> p g c", g=4)      # [128, 4, 65]
        rc = rpool.tile([128, 4], f32, tag="rc", name="rc")
        nc.vector.reciprocal(rc[:], pOr[:, :, 64])
        rcb = bass.AP(
            tensor=rc.tensor, offset=rc.offset,
            ap=[rc.ap[0], rc.ap[1], [0, 64]],
        )
        nc.vector.tensor_mul(
            out_sb[:, 4 * q:4 * q + 4, hsl], pOr[:, :, 0:64], rcb
        )
        if h == 7:
            engs = [nc.sync, nc.gpsimd, nc.scalar, nc.sync]
            for g2 in range(4):
                sl2 = slice(4 * q + g2, 4 * q + g2 + 1)
                engs[g2].dma_start(out=outr[:, sl2, hsl], in_=out_sb[:, sl2, hsl])
        else:
            eng = nc.sync if q == 0 else nc.gpsimd
            eng.dma_start(
                out=outr[:, 4 * q:4 * q + 4, hsl],
                in_=out_sb[:, 4 * q:4 * q + 4, hsl],
            )

    # schedule: (head, half) S-emissions and (head, quad) O-emissions are
    # interleaved one step apart; QKV projections dropped in just before the
    # first S-half that needs them. V only feeds O so it comes after S(0).
    qk_proj(0, "q")
    qk_proj(0, "k")
    emit_S_half(0, 0)
    for m in range(4):
        v_proj(m)
    for h in range(2, 5):
        nc.vector.memset(kTz[h][:], 0.0)
    emit_S_half(0, 1)
    for m in range(4, 8):
        v_proj(m)
    for h in range(5, 8):
        nc.vector.memset(kTz[h][:], 0.0)
    # qk pieces for projection j are spread across the 4 steps of head block
    # 2j-1 so they never bunch up in front of an S-fill.
    inject = {
        (1, i): (1, w, l2) for i, (w, l2) in enumerate(
            [("q", 0), ("q", 1), ("k", 0), ("k", 1)])
    }
    inject.update({(3, i): (2, w, l2) for i, (w, l2) in enumerate(
        [("q", 0), ("q", 1), ("k", 0), ("k", 1)])})
    inject.update({(5, i): (3, w, l2) for i, (w, l2) in enumerate(
        [("q", 0), ("q", 1), ("k", 0), ("k", 1)])})
    for h in range(1, 8):
        for stepi, (kind, hh, part) in enumerate(
            [("S", h, 0), ("O", h - 1, 0), ("S", h, 1), ("O", h - 1, 1)]
        ):
            if kind == "S":
                emit_S_half(hh, part)
            else:
                emit_O_quad(hh, part)
            if (h, stepi) in inject:
                j, w, l2 = inject[(h, stepi)]
                qk_piece(j, w, l2)
    emit_O_quad(7, 0)
    flush_exp()
    emit_O_quad(7, 1)
    ctx.close()


def _build():
    if "nc" in _CACHE:
        return _CACHE["nc"]
    nc = bacc.Bacc("TRN2", target_bir_lowering=False, debug=False, num_devices=8)
    aps = {
        "x": nc.dram_tensor("x", [L, D], bf16, kind="ExternalInput").ap(),
        "wq": nc.dram_tensor("wq", [D, J], bf16, kind="ExternalInput").ap(),
        "wk": nc.dram_tensor("wk", [D, J], bf16, kind="ExternalInput").ap(),
        "wv": nc.dram_tensor("wv", [D, J], bf16, kind="ExternalInput").ap(),
        "bqc": nc.dram_tensor("bqc", [128, 4], f32, kind="ExternalInput").ap(),
        "pet": nc.dram_tensor("pet", [D, L], bf16, kind="ExternalInput").ap(),
        "out": nc.dram_tensor("out", [L, J], f32, kind="ExternalOutput").ap(),
    }
    with tile.TileContext(nc) as tc:
        _emit(tc, aps)
    nc.compile()
    _CACHE["nc"] = nc
    return nc


def _pe_T():
    embed = np.arange(L, dtype=np.float32)
    dim_t = np.arange(D, dtype=np.float32)
    dim_t = (np.float32(TEMPERATURE) ** (2.0 * np.floor(dim_t / 2.0) / np.float32(D))).astype(np.float32)
    pos = embed[:, None] / dim_t  # [L, D]
    pe = np.stack([np.sin(pos[:, 0::2]), np.cos(pos[:, 1::2])], axis=2).reshape(L, D)
    return np.ascontiguousarray(pe.T.astype(np.float32))  # [D, L]


def kernel(**inputs):
    global LAST_RESULT
    bf = np.float16
    x = np.asarray(inputs["x"], dtype=np.float32).astype(bf)
    wq = np.ascontiguousarray(np.asarray(inputs["Wq"], dtype=np.float32).astype(bf))
    wk = np.ascontiguousarray(np.asarray(inputs["Wk"], dtype=np.float32).astype(bf))
    wv = np.ascontiguousarray(np.asarray(inputs["Wv"], dtype=np.float32).astype(bf))
    bq = np.asarray(inputs["bq"], dtype=np.float32)
    bv = np.asarray(inputs["bv"], dtype=np.float32)

    nc = _build()
    bqc = np.ascontiguousarray(np.repeat(bq, HD).reshape(4, 128).T)  # [128, 4]
    pet = _pe_T().astype(bf)
    base = {"wq": wq, "wk": wk, "wv": wv, "bqc": bqc, "pet": pet}
    in_maps = [{**base, "x": np.ascontiguousarray(x[b])} for b in range(B)]
    res = run_bass_kernel_spmd(
        nc, in_maps, core_ids=list(range(B)), trace=TRACE
    )
    LAST_RESULT = res
    out = np.stack([res.results[b]["out"] for b in range(B)]).astype(np.float32)
    out += np.repeat(bv, HD)[None, None, :]
    return out


# revision 20
# speedup vs baseline: 1.0150x; 1.0150x over previous
"""Multi-head distance (attention) layer on 8 TRN2 NeuronCores.

Sharding: data-parallel over batch. B=8 -> one batch element per core.
Each core computes a full multi-head self-attention for its [L=1024, D=256]
slice with H=8 heads of dim 64. No collectives needed.

Per-core algorithm (all layouts chosen so softmax needs no transposes and
all matmul operands are bf16 so the PE streams at 1 row/cycle):
  xT   = transpose(x)            (PE matmul against identity, ACT drains)
  qkT  = xT + peT                (pos-enc, host-precomputed constant, DVE)
  qT   = Wq.T @ x_pe             via matmul(lhsT=Wq, rhs=qkT)
  kTz  = Wk.T @ x_pe             per-head tiles, other head's rows zeroed
                                 (so S contracts K=128: K=64 runs half-rate)
  v    = x @ Wv                  via matmul(lhsT=xT, rhs=Wv)
  per head h:
    sT[m,l] = sum_d kTz[d,m] qT[d,l]     matmul, K=128 (zero-padded)
    eT      = exp(0.125 * sT)            ScalarE, PSUM->SBUF, bf16; S-chunks
                                         packed 3-per-PSUM-tile so each exp
                                         call is [128,1536]
    O[l,d]+Z = eT.T @ [v_h | 1]          matmul(lhsT=eT, rhs=v_aug), 4 output
                                         column-groups share one PSUM bank
    out_h   = O * (1/Z)                  DVE reciprocal + broadcast multiply
Bias handling: bq added to qT during PSUM drain (per-partition scalar, fp32
before bf16 rounding); bk only shifts each score row by a constant
(softmax-invariant) so it is dropped; bv shifts the output by exactly
repeat(bv, 64) because softmax rows sum to 1, added on the host.
"""

import numpy as np
import ml_dtypes

import concourse.bass as bass
import concourse.mybir as mybir
import concourse.tile as tile
from concourse import bacc
from concourse.bass_utils import run_bass_kernel_spmd
from concourse.masks import make_identity

B, L, D = 8, 1024, 256
H, HD = 8, 64
J = H * HD  # 512
TEMPERATURE = 10000.0

f32 = mybir.dt.float32
bf16 = mybir.dt.float16  # fp16: same PE rate as bf16, 8x the mantissa

_CACHE = {}
LAST_RESULT = None  # BassKernelResults of the most recent run (for profiling)
TRACE = False

STILE = 1024  # S-chunk PSUM/exp tile width (2 chunks of 512)


def _emit(tc, aps):
    nc = tc.nc
    Exp = mybir.ActivationFunctionType.Exp
    Copy = mybir.ActivationFunctionType.Copy
    x, wq, wk, wv, bqc, pet, out = (
        aps["x"], aps["wq"], aps["wk"], aps["wv"], aps["bqc"], aps["pet"], aps["out"],
    )

    xr = x.rearrange("(n p) c -> p n c", p=128)          # [128, 8, 256]
    petr = pet.rearrange("(t p) l -> t p l", p=128)      # [2, 128, 1024]
    wqr = wq.rearrange("(t p) j -> t p j", p=128)        # [2, 128, 512]
    wkr = wk.rearrange("(t p) j -> t p j", p=128)
    wvr = wv.rearrange("(t p) j -> t p j", p=128)
    outr = out.rearrange("(n p) j -> p n j", p=128)      # [128, 8, 512]

    import contextlib
    ctx = contextlib.ExitStack()
    persist = ctx.enter_context(tc.tile_pool(name="persist", bufs=1))
    epool = ctx.enter_context(tc.tile_pool(name="epool", bufs=18))
    rpool = ctx.enter_context(tc.tile_pool(name="rpool", bufs=4))
    s_ps = ctx.enter_context(tc.tile_pool(name="sps", bufs=3, space="PSUM"))
    o_ps = ctx.enter_context(tc.tile_pool(name="ops", bufs=2, space="PSUM"))

    # --- ACT exp-table preload (off the attention critical path) ---
    sc_in = persist.tile([128, 8], f32, name="sc_in")
    sc_out = persist.tile([128, 8], f32, name="sc_out")
    nc.vector.memset(sc_in[:], 0.0)
    nc.scalar.activation(sc_out[:], sc_in[:], Exp)

    kTz = [persist.tile([128, 1024], bf16, name=f"kTz{h}") for h in range(8)]
    for h in range(2):
        nc.vector.memset(kTz[h][:], 0.0)

    # --- input DMAs: x + wq on the SP HWDGE queue, rest on gpsimd SWDGE ---
    x_sb = persist.tile([128, 8, 256], bf16, name="x_sb")
    for qtr in range(4):
        nc.sync.dma_start(out=x_sb[:, qtr * 2:(qtr + 1) * 2, :],
                          in_=xr[:, qtr * 2:(qtr + 1) * 2, :])
    w_sb = {}
    for wname in ("wq", "wk", "wv"):
        w_sb[wname] = [
            persist.tile([128, 512], bf16, name=f"{wname}_sb{t}") for t in range(2)
        ]
    for t in range(2):
        nc.sync.dma_start(out=w_sb["wq"][t][:], in_=wqr[t])

    ident = persist.tile([128, 128], bf16, name="ident")
    make_identity(nc, ident)

    bq_sb = persist.tile([128, 4], f32, name="bq_sb")
    nc.gpsimd.dma_start(out=bq_sb[:], in_=bqc[:, :])
    pe_sb = [persist.tile([128, 1024], bf16, name=f"pe_sb{t}") for t in range(2)]
    for t in range(2):
        nc.gpsimd.dma_start(out=pe_sb[t][:], in_=petr[t])
    for wname, wr in (("wk", wkr), ("wv", wvr)):
        for t in range(2):
            nc.gpsimd.dma_start(out=w_sb[wname][t][:], in_=wr[t])

    # --- transpose x via PE (out = x_chunk.T @ I); 4 transposes packed per
    # PSUM tile, drains split between ScalarE and DVE ---
    xT = [persist.tile([128, 1024], bf16, name=f"xT{t}") for t in range(2)]
    for c2 in range(2):
        for g in range(2):  # n-groups of 4
            tp = s_ps.tile([128, STILE], f32, tag="s", name="tp")
            for i in range(4):
                n = 4 * g + i
                nc.tensor.matmul(
                    tp[:, i * 128:(i + 1) * 128],
                    lhsT=x_sb[:, n, c2 * 128:(c2 + 1) * 128],
                    rhs=ident[:],
                    start=True,
                    stop=True,
                )
            dst = xT[c2][:, g * 512:(g + 1) * 512]
            if g == 0:
                nc.scalar.activation(dst, tp[:, 0:512], Copy)
            else:
                nc.vector.tensor_copy(dst, tp[:, 0:512])

    qkT = [persist.tile([128, 1024], bf16, name=f"qkT{t}") for t in range(2)]
    for t in range(2):
        nc.vector.tensor_add(qkT[t][:], xT[t][:], pe_sb[t][:])

    # --- QKV projections (PSUM fills share the "s" tag slots) ---
    qT = [persist.tile([128, 1024], bf16, name=f"qT{j}") for j in range(4)]
    v_sb = [persist.tile([128, 8, 65], bf16, name=f"v_sb{m}") for m in range(8)]

    def qk_piece(j, which, l2):
        wname = "wq" if which == "q" else "wk"
        pq = s_ps.tile([128, STILE], f32, tag="s", name="pq")
        for c2 in range(2):
            nc.tensor.matmul(
                pq[:, 0:512],
                lhsT=w_sb[wname][c2][:, j * 128:(j + 1) * 128],
                rhs=qkT[c2][:, l2 * 512:(l2 + 1) * 512],
                start=(c2 == 0),
                stop=(c2 == 1),
            )
        dsl = slice(l2 * 512, (l2 + 1) * 512)
        if which == "q":
            nc.vector.tensor_scalar_add(
                qT[j][:, dsl], pq[:, 0:512], bq_sb[:, j:j + 1]
            )
        else:
            nc.vector.tensor_copy(kTz[2 * j][0:64, dsl], pq[0:64, 0:512])
            nc.vector.tensor_copy(kTz[2 * j + 1][64:128, dsl], pq[64:128, 0:512])

    def qk_proj(j, which):
        for l2 in range(2):
            qk_piece(j, which, l2)

    def v_proj(m):
        pv = s_ps.tile([128, STILE], f32, tag="s", name="pv")
        for c2 in range(2):
            nc.tensor.matmul(
                pv[:, 0:512],
                lhsT=xT[c2][:, m * 128:(m + 1) * 128],
                rhs=w_sb["wv"][c2][:],
                start=(c2 == 0),
                stop=(c2 == 1),
            )
        nc.vector.tensor_copy(
            v_sb[m][:, :, 0:64], pv[:, 0:512].rearrange("p (h d) -> p h d", h=8)
        )
        nc.vector.memset(v_sb[m][:, :, 64:65], 1.0)

    # --- attention: S-chunks packed into [128, STILE] PSUM tiles; one exp
    # per tile. Software-pipelined: S(h+1) emitted before O(h). ---
    out_sb = persist.tile([128, 8, 512], f32, name="out_sb")
    epos = {}  # (h, mc, l2) -> (e_tile, col_offset)
    state = {"tile": None, "off": 0, "chunks": []}

    def flush_exp():
        if state["tile"] is None or not state["chunks"]:
            return
        e = epool.tile([128, state["off"]], bf16, tag="e", name="e")
        nc.scalar.activation(
            e[:], state["tile"][:, 0:state["off"]], Exp, scale=float(HD) ** -0.5
        )
        for key, off in state["chunks"]:
            epos[key] = (e, off)
        state["tile"] = None
        state["off"] = 0
        state["chunks"] = []

    def s_chunk(h, mc, l2):
        if state["tile"] is None:
            state["tile"] = s_ps.tile([128, STILE], f32, tag="s", name="ps")
        off = state["off"]
        nc.tensor.matmul(
            state["tile"][:, off:off + 512],
            lhsT=kTz[h][:, mc * 128:(mc + 1) * 128],
            rhs=qT[h // 2][:, l2 * 512:(l2 + 1) * 512],
            start=True,
            stop=True,
        )
        state["chunks"].append(((h, mc, l2), off))
        state["off"] = off + 512
        if state["off"] == STILE:
            flush_exp()

    def emit_S_half(h, l2):
        for mc in range(8):
            s_chunk(h, mc, l2)

    def emit_O_quad(h, q):
        hsl = slice(h * 64, (h + 1) * 64)
        pO = o_ps.tile([128, 260], f32, tag="o", name="pO")
        for g in range(4):
            lc = 4 * q + g
            l2, sub = lc // 4, lc % 4
            for mc in range(8):
                e, off = epos[(h, mc, l2)]
                nc.tensor.matmul(
                    pO[:, 65 * g:65 * g + 65],
                    lhsT=e[:, off + sub * 128:off + (sub + 1) * 128],
                    rhs=v_sb[mc][:, h, :],
                    start=(mc == 0),
                    stop=(mc == 7),
                )
        pOr = pO.rearrange("p (g c) -> p g c", g=4)      # [128, 4, 65]
        rc = rpool.tile([128, 4], f32, tag="rc", name="rc")
        nc.vector.reciprocal(rc[:], pOr[:, :, 64])
        rcb = bass.AP(
            tensor=rc.tensor, offset=rc.offset,
            ap=[rc.ap[0], rc.ap[1], [0, 64]],
        )
        nc.vector.tensor_mul(
            out_sb[:, 4 * q:4 * q + 4, hsl], pOr[:, :, 0:64], rcb
        )
        if h == 7:
            engs = [nc.sync, nc.gpsimd, nc.scalar, nc.sync]
            for g2 in range(4):
                sl2 = slice(4 * q + g2, 4 * q + g2 + 1)
                engs[g2].dma_start(out=outr[:, sl2, hsl], in_=out_sb[:, sl2, hsl])
        else:
            eng = nc.sync if q == 0 else nc.gpsimd
            eng.dma_start(
                out=outr[:, 4 * q:4 * q + 4, hsl],
                in_=out_sb[:, 4 * q:4 * q + 4, hsl],
            )

    # schedule: (head, half) S-emissions and (head, quad) O-emissions are
    # interleaved one step apart; QKV projections dropped in just before the
    # first S-half that needs them. V only feeds O so it comes after S(0).
    qk_proj(0, "q")
    qk_proj(0, "k")
    emit_S_half(0, 0)
    for m in range(4):
        v_proj(m)
    for h in range(2, 5):
        nc.vector.memset(kTz[h][:], 0.0)
    emit_S_half(0, 1)
    for m in range(4, 8):
        v_proj(m)
    for h in range(5, 8):
        nc.vector.memset(kTz[h][:], 0.0)
    # qk pieces for projection j are spread across the 4 steps of head block
    # 2j-1 so they never bunch up in front of an S-fill.
    inject = {
        (1, i): (1, w, l2) for i, (w, l2) in enumerate(
            [("q", 0), ("q", 1), ("k", 0), ("k", 1)])
    }
    inject.update({(3, i): (2, w, l2) for i, (w, l2) in enumerate(
        [("q", 0), ("q", 1), ("k", 0), ("k", 1)])})
    inject.update({(5, i): (3, w, l2) for i, (w, l2) in enumerate(
        [("q", 0), ("q", 1), ("k", 0), ("k", 1)])})
    for h in range(1, 8):
        for stepi, (kind, hh, part) in enumerate(
            [("S", h, 0), ("O", h - 1, 0), ("S", h, 1), ("O", h - 1, 1)]
        ):
            if kind == "S":
                emit_S_half(hh, part)
            else:
                emit_O_quad(hh, part)
            if (h, stepi) in inject:
                j, w, l2 = inject[(h, stepi)]
                qk_piece(j, w, l2)
    emit_O_quad(7, 0)
    flush_exp()
    emit_O_quad(7, 1)
    ctx.close()


def _build():
    if "nc" in _CACHE:
        return _CACHE["nc"]
    nc = bacc.Bacc("TRN2", target_bir_lowering=False, debug=False, num_devices=8)
    aps = {
        "x": nc.dram_tensor("x", [L, D], bf16, kind="ExternalInput").ap(),
        "wq": nc.dram_tensor("wq", [D, J], bf16, kind="ExternalInput").ap(),
        "wk": nc.dram_tensor("wk", [D, J], bf16, kind="ExternalInput").ap(),
        "wv": nc.dram_tensor("wv", [D, J], bf16, kind="ExternalInput").ap(),
        "bqc": nc.dram_tensor("bqc", [128, 4], f32, kind="ExternalInput").ap(),
        "pet": nc.dram_tensor("pet", [D, L], bf16, kind="ExternalInput").ap(),
        "out": nc.dram_tensor("out", [L, J], f32, kind="ExternalOutput").ap(),
    }
    with tile.TileContext(nc) as tc:
        _emit(tc, aps)
    nc.compile()
    _CACHE["nc"] = nc
    return nc


def _pe_T():
    embed = np.arange(L, dtype=np.float32)
    dim_t = np.arange(D, dtype=np.float32)
    dim_t = (np.float32(TEMPERATURE) ** (2.0 * np.floor(dim_t / 2.0) / np.float32(D))).astype(np.float32)
    pos = embed[:, None] / dim_t  # [L, D]
    pe = np.stack([np.sin(pos[:, 0::2]), np.cos(pos[:, 1::2])], axis=2).reshape(L, D)
    return np.ascontiguousarray(pe.T.astype(np.float32))  # [D, L]


def kernel(**inputs):
    global LAST_RESULT
    bf = np.float16
    x = np.asarray(inputs["x"], dtype=np.float32).astype(bf)
    wq = np.ascontiguousarray(np.asarray(inputs["Wq"], dtype=np.float32).astype(bf))
    wk = np.ascontiguousarray(np.asarray(inputs["Wk"], dtype=np.float32).astype(bf))
    wv = np.ascontiguousarray(np.asarray(inputs["Wv"], dtype=np.float32).astype(bf))
    bq = np.asarray(inputs["bq"], dtype=np.float32)
    bv = np.asarray(inputs["bv"], dtype=np.float32)

    nc = _build()
    bqc = np.ascontiguousarray(np.repeat(bq, HD).reshape(4, 128).T)  # [128, 4]
    pet = _pe_T().astype(bf)
    base = {"wq": wq, "wk": wk, "wv": wv, "bqc": bqc, "pet": pet}
    in_maps = [{**base, "x": np.ascontiguousarray(x[b])} for b in range(B)]
    res = run_bass_kernel_spmd(
        nc, in_maps, core_ids=list(range(B)), trace=TRACE
    )
    LAST_RESULT = res
    out = np.stack([res.results[b]["out"] for b in range(B)]).astype(np.float32)
    out += np.repeat(bv, HD)[None, None, :]
    return out
